# revision 19
# baseline (speedup 1.0000x reference)
"""Trainium2 Bass kernel for the FCBlock weight-transform + matmul problem.

Math (per reference):
    W_i = per-head 3x3 conv over W.reshape(4, 1024, 4096) + conv_b
          + sigmoid(sk_wt) * W            (per-head scalars)
    out  = inp @ W_i.T                    (inp: [2, 2048, 4096])

Strategy: tensor-parallel shard of W along fout across 8 NeuronCores
(512 rows each; each core's shard lies within one head).  The main
matmul runs in fp8 (e4m3) with the tensor engine's DoubleRow perf mode
(K=256 per instruction, 2x bf16 throughput).  fp8 quantization of W_i
alone would miss the accuracy gate because W_i is dominated by the
per-head conv bias, so the kernel uses a rank-1 mean-compensation: the
per-row mean mu[o] of W_i is subtracted before fp8 quantization
(folded into the conv-bias add) and the exact rank-1 term S[t]*mu[o]
(S = f32 row sums of inp) is added back during the output evacuation.

The weight transform runs directly in TRANSPOSED space: the host ships
W^T (bf16) sliced into 1-row-shifted 128-fin-row windows, so the conv
becomes, per fin window, 3 banded bf16 PE matmuls (one per fout shift
dr, fin shifts dc carried by the band diagonals) plus one 6-row halo
matmul for the 2 fin rows past the window edge.  PSUM evacuation adds
the per-fout-column offset (conv_b - mu) and casts straight to the fp8
W_i^T tile -- no on-device transposes at all.

Host side also pre-transposes/pre-casts x to fp8 x^T in k-block layout
(so the 64MB activation tensor needs no on-device transpose/cast) and
replicates S across partitions.  The device output is out^T
[fout_shard, tok]; the host transposes/concatenates shards.
"""

import numpy as np
import ml_dtypes

import concourse.bass as bass
import concourse.mybir as mybir
import concourse.tile as tile
from concourse import bacc
from concourse.bass_utils import run_bass_kernel_spmd

F32 = mybir.dt.float32
BF16 = mybir.dt.bfloat16
F8 = mybir.dt.float8e4

NP_F8 = ml_dtypes.float8_e4m3
NP_BF16 = ml_dtypes.bfloat16

NCORES = 8
NUM_HEADS = 4
TOK = 4096          # 2 * 2048 tokens
FIN = 4096
FOUT = 4096
FSH = FOUT // NCORES  # 512 fout rows per core
N_K = FIN // 128      # 32 contraction blocks
N_WIN = FSH // 128    # 4 output chunks per core
N_SLOT = N_K          # 32 shifted fin windows of W^T
WTW = FSH + 2         # 514 fout columns per W^T slot (1-col halo each side)
SLOT_CHUNKS = ((0, 4), (4, 6), (10, 6), (16, 6), (22, 6), (28, 4))
HAL_CHUNKS = 8        # halo rows streamed in 8 chunks of 4 windows


def build_program(tok=TOK, probe=()):
    """Build the per-core SPMD program (tok parameterized for mini tests)."""
    assert tok % 512 == 0
    n_kp = N_K // 2          # 16 DoubleRow k-pairs
    n_tb = tok // 512        # token blocks of 512 (one PSUM bank each)

    nc = bacc.Bacc(None, target_bir_lowering=False)

    xt_d = nc.declare_dram_parameter("xt", [128, N_K * tok], F8, isOutput=False)
    wht = nc.declare_dram_parameter("wht", [128, N_SLOT * WTW], BF16,
                                    isOutput=False)
    hal = nc.declare_dram_parameter("hal", [6, N_K * 512], BF16, isOutput=False)
    sc = nc.declare_dram_parameter("sc", [1, 11], F32, isOutput=False)
    aux = nc.declare_dram_parameter("aux", [128, N_WIN], F32, isOutput=False)
    offr = nc.declare_dram_parameter("offr", [128, FSH], F32, isOutput=False)
    srep = nc.declare_dram_parameter("srep", [128, tok], F32, isOutput=False)
    out = nc.declare_dram_parameter("o", [FSH, tok], F32, isOutput=True)

    DR = mybir.MatmulPerfMode.DoubleRow

    with tile.TileContext(nc) as tc:
        with (
            tc.tile_pool(name="const", bufs=1) as const,
            tc.tile_pool(name="wtpool", bufs=1) as wtpool,
            tc.tile_pool(name="xtp", bufs=1) as xtp,
            tc.tile_pool(name="ssb", bufs=1) as ssbp,
            tc.tile_pool(name="whp", bufs=3) as whp,
            tc.tile_pool(name="hfp", bufs=3) as hfp,
            tc.tile_pool(name="ob", bufs=3) as obp,
            tc.tile_pool(name="psw", bufs=4, space="PSUM") as psw,
            tc.tile_pool(name="psx", bufs=4, space="PSUM") as psx,
        ):
            # ---- small inputs (ACT hwdge ring, ahead of everything) -------
            sc_sb = const.tile([1, 11], F32)
            nc.scalar.dma_start(out=sc_sb[:], in_=sc[:])
            aux_sb = const.tile([128, N_WIN], F32)
            nc.scalar.dma_start(out=aux_sb[:], in_=aux[:])
            offr_sb = const.tile([128, FSH], F32)
            nc.scalar.dma_start(out=offr_sb[:], in_=offr[:])

            # ---- W^T slot chunk tiles, alternating between the sync and
            # ACT hwdge rings (each ring sustains only ~230GB/s for a single
            # transfer; two parallel streams keep the transform fed).  All 6
            # are queued up-front; the 3-buffer pool gates chunks 3-5 on
            # transform progress (monotone -> deadlock-free). ---------------
            wh_tiles = []
            for ci, (s0, ns) in enumerate(SLOT_CHUNKS):
                wch = whp.tile([128, ns, WTW], BF16, tag="wch", name=f"wch{ci}")
                wh_tiles.append((s0, ns, wch))
                eng = nc.sync if ci % 2 == 0 else nc.scalar
                eng.dma_start(out=wch[:], in_=wht[:, s0 * WTW:(s0 + ns) * WTW])

            # ---- halo-row chunk tiles (pre-packed [6, v, 512] on host) on
            # the gpsimd ring, where their PE-progress gates stall nothing --
            hal_tiles = []
            for ci in range(HAL_CHUNKS):
                hch = hfp.tile([6, 4, 512], BF16, tag="hal", name=f"hal{ci}")
                hal_tiles.append(hch)
                if ci < 3:
                    nc.gpsimd.dma_start(
                        out=hch[:],
                        in_=hal[:, 4 * ci * 512:4 * (ci + 1) * 512])

            # ---- bulk x^T load, split across both hwdge rings; S-replica
            # (needed by the first phase-M evacuation) lands on the ACT ring
            # before the last two x^T chunks --------------------------------
            xt = xtp.tile([128, N_K, tok], F8)
            s_sb = ssbp.tile([128, tok], F32)
            for j in range(4):
                nc.sync.dma_start(
                    out=xt[:, 4 * j:4 * j + 4, :],
                    in_=xt_d[:, 4 * j * tok:(4 * j + 4) * tok])
            for j in (4, 5):
                nc.scalar.dma_start(
                    out=xt[:, 4 * j:4 * j + 4, :],
                    in_=xt_d[:, 4 * j * tok:(4 * j + 4) * tok])
            nc.scalar.dma_start(out=s_sb[:], in_=srep[:])
            for j in (6, 7):
                nc.scalar.dma_start(
                    out=xt[:, 4 * j:4 * j + 4, :],
                    in_=xt_d[:, 4 * j * tok:(4 * j + 4) * tok])

            # ---- setup: broadcast scalars, band + halo matrices -----------
            ones_r = const.tile([1, 128], F32)
            nc.vector.memset(ones_r[:], 1.0)
            ps_b = psw.tile([128, 11], F32, tag="pw")
            nc.tensor.matmul(ps_b[:], ones_r[:], sc_sb[:], start=True, stop=True)
            scv = const.tile([128, 11], F32)
            nc.vector.tensor_copy(out=scv[:], in_=ps_b[:])

            # ctr = conv_w[h,1,1] + sigmoid(sk_wt[h])
            sig = const.tile([128, 1], F32)
            nc.scalar.activation(sig[:], scv[:, 10:11],
                                 mybir.ActivationFunctionType.Sigmoid)
            ctr = const.tile([128, 1], F32)
            nc.vector.tensor_tensor(out=ctr[:], in0=sig[:], in1=scv[:, 4:5],
                                    op=mybir.AluOpType.add)

            # band matrices B_dr[k, f] = cw[h, dr, k-f] (k-f = dc in {0,1,2});
            # the dr=1 matmul's center diagonal also carries the sigmoid
            # residual.  (Transposed space: contraction = shifted fin rows.)
            masks = []
            for d in range(3):
                m = const.tile([128, 128], F32, tag=f"mask{d}")
                nc.gpsimd.memset(m[:], 0.0)
                nc.gpsimd.affine_select(
                    out=m[:], in_=m[:],
                    compare_op=mybir.AluOpType.not_equal,
                    fill=1.0, base=-d, channel_multiplier=1,
                    pattern=[[-1, 128]],
                )
                masks.append(m)
            b_bf = []
            for dr in range(3):
                bf_ = const.tile([128, 128], F32, tag=f"bf_{dr}")
                nc.vector.tensor_scalar(bf_[:], masks[0][:],
                                        scv[:, 3 * dr:3 * dr + 1],
                                        None, mybir.AluOpType.mult)
                mid = ctr if dr == 1 else scv[:, 3 * dr + 1:3 * dr + 2]
                nc.vector.scalar_tensor_tensor(
                    out=bf_[:], in0=masks[1][:], scalar=mid, in1=bf_[:],
                    op0=mybir.AluOpType.mult, op1=mybir.AluOpType.add)
                nc.vector.scalar_tensor_tensor(
                    out=bf_[:], in0=masks[2][:],
                    scalar=scv[:, 3 * dr + 2:3 * dr + 3],
                    in1=bf_[:],
                    op0=mybir.AluOpType.mult, op1=mybir.AluOpType.add)
                bb = const.tile([128, 128], BF16, tag=f"bb_{dr}")
                nc.vector.tensor_copy(out=bb[:], in_=bf_[:])
                b_bf.append(bb)

            # halo matrix H6 [6, 128], j = row*3+dr over the 2 halo fin rows
            # x 3 fout shifts: col 127 takes center/dc=+1 taps from halo rows
            # 0/1, col 126 its dc=+1 tap from halo row 0.  Built as outer
            # products (engine APs cannot start at a nonzero partition).
            onehot = const.tile([1, 128], F32)
            nc.vector.memset(onehot[:], 0.0)
            nc.vector.memset(onehot[:, 127:128], 1.0)
            onehot6 = const.tile([1, 128], F32)
            nc.vector.memset(onehot6[:], 0.0)
            nc.vector.memset(onehot6[:, 126:127], 1.0)
            sig0 = const.tile([1, 1], F32)
            nc.scalar.activation(sig0[:], sc_sb[:, 10:11],
                                 mybir.ActivationFunctionType.Sigmoid)
            ctr0 = const.tile([1, 1], F32)
            nc.vector.tensor_tensor(out=ctr0[:], in0=sig0[:], in1=sc_sb[:, 4:5],
                                    op=mybir.AluOpType.add)
            # v1 (col 127): row0 -> cw[dr,1] (ctr at dr=1), row1 -> cw[dr,2]
            v1 = const.tile([1, 6], F32)
            nc.vector.tensor_copy(out=v1[:, 0:1], in_=sc_sb[:, 1:2])
            nc.vector.tensor_copy(out=v1[:, 1:2], in_=ctr0[:])
            nc.vector.tensor_copy(out=v1[:, 2:3], in_=sc_sb[:, 7:8])
            nc.vector.tensor_copy(out=v1[:, 3:4], in_=sc_sb[:, 2:3])
            nc.vector.tensor_copy(out=v1[:, 4:5], in_=sc_sb[:, 5:6])
            nc.vector.tensor_copy(out=v1[:, 5:6], in_=sc_sb[:, 8:9])
            # v2 (col 126): row0 -> cw[dr,2]
            v2 = const.tile([1, 6], F32)
            nc.vector.memset(v2[:], 0.0)
            nc.vector.tensor_copy(out=v2[:, 0:1], in_=sc_sb[:, 2:3])
            nc.vector.tensor_copy(out=v2[:, 1:2], in_=sc_sb[:, 5:6])
            nc.vector.tensor_copy(out=v2[:, 2:3], in_=sc_sb[:, 8:9])
            ph = psw.tile([6, 128], F32, tag="pw")
            nc.tensor.matmul(ph[:], v1[:], onehot[:], start=True, stop=False)
            nc.tensor.matmul(ph[:], v2[:], onehot6[:], start=False, stop=True)
            h6 = const.tile([6, 128], BF16)
            nc.vector.tensor_copy(out=h6[:], in_=ph[:])

            wt = wtpool.tile([128, N_K, FSH], F8)      # W_i^T, fin-major, fp8

            # ---- phase T: transposed weight transform ---------------------
            for ci, (s0, ns, wch) in enumerate(wh_tiles):
                for vi in range(ns):
                    v = s0 + vi
                    if v % 4 == 0 and 3 <= v // 4 + 2 < HAL_CHUNKS:
                        c2 = v // 4 + 2
                        nc.gpsimd.dma_start(
                            out=hal_tiles[c2][:],
                            in_=hal[:, 4 * c2 * 512:4 * (c2 + 1) * 512])
                    pw = psw.tile([128, 512], F32, tag="pw")
                    for dr in range(3):
                        nc.tensor.matmul(
                            pw[:], b_bf[dr][:], wch[:, vi, dr:dr + 512],
                            start=(dr == 0), stop=False)
                    nc.tensor.matmul(pw[:], h6[:],
                                     hal_tiles[v // 4][:, v % 4, :],
                                     start=False, stop=True)
                    # PSUM -> fp8 W_i^T slot with (conv_b - mu)[o] column add
                    nc.vector.tensor_tensor(
                        out=wt[:, v, :], in0=pw[:], in1=offr_sb[:],
                        op=mybir.AluOpType.add)

            # ---- phase M: fp8 DoubleRow matmul, W_i^T chunks stationary ---
            # out^T[oc*128 + p, t]; k-inner per PSUM bank (no bank switch
            # between accumulating matmuls).
            for oc in range(N_WIN):
                pos = []
                for b in range(n_tb):
                    pool, tg = (psw, "pw") if b % 2 == 0 else (psx, "px")
                    pos.append(pool.tile([128, 512], F32, tag=tg,
                                         name=f"po{b}"))
                for b in range(n_tb):
                    for k in range(n_kp):
                        if "no_mm" in probe:
                            continue
                        nc.tensor.matmul(
                            pos[b][:],
                            wt[:, 2 * k:2 * k + 2, 128 * oc:128 * oc + 128],
                            xt[:, 2 * k:2 * k + 2, 512 * b:512 * b + 512],
                            start=(k == 0), stop=(k == n_kp - 1),
                            perf_mode=DR)
                    if "no_mm" in probe:
                        nc.vector.memset(pos[b][:], 0.0)
                    ob = obp.tile([128, 512], F32, tag="ob")
                    # ob = psum + mu[o] * S[t]  (rank-1 mean compensation)
                    nc.vector.scalar_tensor_tensor(
                        out=ob[:], in0=s_sb[:, 512 * b:512 * b + 512],
                        scalar=aux_sb[:, oc:oc + 1],
                        in1=pos[b][:],
                        op0=mybir.AluOpType.mult, op1=mybir.AluOpType.add)
                    nc.sync.dma_start(
                        out=out[128 * oc:128 * oc + 128,
                                512 * b:512 * b + 512],
                        in_=ob[:])

    nc.compile()
    return nc


def _host_transform_f32(W, conv_w, conv_b, sk_wt):
    """f32 numpy replica of the reference weight transform (used only to
    derive the per-row means mu for fp8 mean-compensation)."""
    hsz = FOUT // NUM_HEADS
    mat = W.reshape(NUM_HEADS, hsz, FIN)
    sig = (1.0 / (1.0 + np.exp(-sk_wt.astype(np.float64)))).astype(np.float32)
    sig = sig.reshape(NUM_HEADS)
    out = np.empty_like(mat)
    for h in range(NUM_HEADS):
        p = np.pad(mat[h], ((1, 1), (1, 1)))
        acc = np.zeros((hsz, FIN), dtype=np.float32)
        for dr in range(3):
            for dc in range(3):
                acc += conv_w[h, 0, dr, dc] * p[dr:dr + hsz, dc:dc + FIN]
        out[h] = acc + conv_b[h] + sig[h] * mat[h]
    return out.reshape(FOUT, FIN)


def shard_inputs(inp, W, conv_w, conv_b, sk_wt):
    """Build the 8 per-core input maps."""
    x = np.ascontiguousarray(np.asarray(inp, dtype=np.float32).reshape(TOK, FIN))
    W = np.asarray(W, dtype=np.float32)
    conv_w = np.asarray(conv_w, dtype=np.float32)
    conv_b = np.asarray(conv_b, dtype=np.float32).reshape(NUM_HEADS)
    sk_wt = np.asarray(sk_wt, dtype=np.float32)

    # fp8 x^T in k-block layout: xt[p, k, t] = fp8(x[t, 128k + p])
    x8T = np.ascontiguousarray(x.astype(NP_F8).T)          # [fin, tok]
    xt_host = np.ascontiguousarray(
        x8T.reshape(N_K, 128, TOK).transpose(1, 0, 2)).reshape(128, N_K * TOK)

    # exact f32 row sums of x, replicated across partitions
    srow = x.astype(np.float64).sum(axis=1).astype(np.float32)
    srep = np.ascontiguousarray(
        np.broadcast_to(srow[None, :], (128, TOK)))

    # per-row means of W_i (host f32 replica; quantization statistic only)
    W_i = _host_transform_f32(W, conv_w, conv_b, sk_wt)
    mu = W_i.astype(np.float64).mean(axis=1).astype(np.float32)  # [FOUT]

    hsz = FOUT // NUM_HEADS
    in_maps = []
    for c in range(NCORES):
        gr0 = c * FSH
        h = (gr0 // hsz) % NUM_HEADS
        # W^T halo slab: rows = fout gr0-1 .. gr0+512 (zero-padded at the
        # head boundary), cols = all fin.  -> [514, FIN] bf16
        slab = np.zeros((FSH + 2, FIN), dtype=NP_BF16)
        lo = max(gr0 - 1, h * hsz)
        hi = min(gr0 + FSH + 1, (h + 1) * hsz)
        slab[lo - (gr0 - 1):hi - (gr0 - 1), :] = W[lo:hi, :].astype(NP_BF16)
        slabT = np.ascontiguousarray(slab.T)               # [FIN, 514]
        # slots: wht[p, v, :] = W^T row (128v + p - 1), zero at fin edges
        rows = np.zeros((128 * N_SLOT, WTW), dtype=NP_BF16)
        for v in range(N_SLOT):
            r0 = 128 * v - 1
            a = max(r0, 0)
            bnd = min(r0 + 128, FIN)
            if a < bnd:
                rows[128 * v + (a - r0):128 * v + (bnd - r0), :] = slabT[a:bnd]
        whT_host = np.ascontiguousarray(
            rows.reshape(N_SLOT, 128, WTW).transpose(1, 0, 2)
        ).reshape(128, N_SLOT * WTW)
        # halo rows: hal[r*3+dr, v, t] = W^T row (128v + 127 + r), col dr+t
        slabT_pad = np.vstack([slabT, np.zeros((1, WTW), dtype=NP_BF16)])
        hal6 = np.zeros((6, N_K, 512), dtype=NP_BF16)
        ridx = 128 * np.arange(N_K) + 127
        for r in range(2):
            for dr in range(3):
                hal6[r * 3 + dr] = slabT_pad[ridx + r, dr:dr + 512]
        hal_host = np.ascontiguousarray(hal6).reshape(6, N_K * 512)

        scal = np.zeros((1, 11), dtype=np.float32)
        scal[0, :9] = conv_w[h].reshape(9)
        scal[0, 9] = conv_b[h]
        scal[0, 10] = np.float32(sk_wt[h].reshape(()))
        muc = mu[gr0:gr0 + FSH]
        auxm = np.ascontiguousarray(muc.reshape(N_WIN, 128).T)   # [128, 4]
        offr = np.ascontiguousarray(np.broadcast_to(
            (conv_b[h] - muc)[None, :], (128, FSH))).astype(np.float32)
        in_maps.append({"xt": xt_host, "wht": whT_host, "hal": hal_host,
                        "sc": scal, "aux": auxm, "offr": offr, "srep": srep})
    return in_maps


_PROGRAM_CACHE = {}


def _get_program(tok=TOK, probe=()):
    key = (tok, tuple(probe))
    if key not in _PROGRAM_CACHE:
        _PROGRAM_CACHE[key] = build_program(tok, probe)
    return _PROGRAM_CACHE[key]


def kernel(inp, W, conv_w, conv_b, sk_wt):
    nc = _get_program(TOK)
    in_maps = shard_inputs(inp, W, conv_w, conv_b, sk_wt)
    res = run_bass_kernel_spmd(nc, in_maps, list(range(NCORES)))
    outT = np.concatenate([res.results[c]["o"] for c in range(NCORES)],
                          axis=0)                      # [FOUT, TOK]
    return np.ascontiguousarray(outT.T.astype(np.float32)
                                ).reshape(2, TOK // 2, FOUT)


# revision 24
# speedup vs baseline: 1.1925x; 1.1925x over previous
"""Trainium2 Bass kernel for the FCBlock weight-transform + matmul problem.

Math (per reference):
    W_i = per-head 3x3 conv over W.reshape(4, 1024, 4096) + conv_b
          + sigmoid(sk_wt) * W            (per-head scalars)
    out  = inp @ W_i.T                    (inp: [2, 2048, 4096])

Strategy: tensor-parallel shard of W along fout across 8 NeuronCores
(512 rows each; each core's shard lies within one head).  The main
matmul runs in fp8 (e4m3) with the tensor engine's DoubleRow perf mode
(K=256 per instruction, 2x bf16 throughput).  fp8 quantization of W_i
alone would miss the accuracy gate because W_i is dominated by the
per-head conv bias, so the kernel uses a rank-1 mean-compensation: the
per-row mean mu[o] of W_i is subtracted before fp8 quantization
(folded into the conv-bias add) and the exact rank-1 term S[t]*mu[o]
(S = f32 row sums of inp) is added back during the output evacuation.

The weight transform runs directly in TRANSPOSED space: the host ships
W^T (bf16) sliced into 1-row-shifted 128-fin-row windows, so the conv
becomes, per fin window, 3 banded bf16 PE matmuls (one per fout shift
dr, fin shifts dc carried by the band diagonals) plus one 6-row halo
matmul for the 2 fin rows past the window edge.  PSUM evacuation adds
the per-fout-column offset (conv_b - mu) and casts straight to the fp8
W_i^T tile -- no on-device transposes at all.

Host side also pre-transposes/pre-casts x to fp8 x^T in k-block layout
(so the 64MB activation tensor needs no on-device transpose/cast) and
replicates S across partitions.  The device output is out^T
[fout_shard, tok]; the host transposes/concatenates shards.
"""

import numpy as np
import ml_dtypes

import concourse.bass as bass
import concourse.mybir as mybir
import concourse.tile as tile
from concourse import bacc
from concourse.bass_utils import run_bass_kernel_spmd

F32 = mybir.dt.float32
BF16 = mybir.dt.bfloat16
F8 = mybir.dt.float8e4

NP_F8 = ml_dtypes.float8_e4m3
NP_BF16 = ml_dtypes.bfloat16

NCORES = 8
NUM_HEADS = 4
TOK = 4096          # 2 * 2048 tokens
FIN = 4096
FOUT = 4096
FSH = FOUT // NCORES  # 512 fout rows per core
N_K = FIN // 128      # 32 contraction blocks
N_WIN = FSH // 128    # 4 output chunks per core
N_SLOT = N_K          # 32 shifted fin windows of W^T
WTW = FSH + 2         # 514 fout columns per W^T slot (1-col halo each side)
SLOT_CHUNKS = ((0, 2), (2, 4), (6, 4), (10, 4), (14, 4), (18, 4), (22, 5),
               (27, 5))
HAL_CHUNKS = 8        # halo rows streamed in 8 chunks of 4 windows


def build_program(tok=TOK, probe=()):
    """Build the per-core SPMD program (tok parameterized for mini tests)."""
    assert tok % 512 == 0
    n_kp = N_K // 2          # 16 DoubleRow k-pairs
    n_tb = tok // 512        # token blocks of 512 (one PSUM bank each)

    nc = bacc.Bacc(None, target_bir_lowering=False)

    xt_d = nc.declare_dram_parameter("xt", [128, N_K * tok], F8, isOutput=False)
    wht = nc.declare_dram_parameter("wht", [128, N_SLOT * WTW], BF16,
                                    isOutput=False)
    hal = nc.declare_dram_parameter("hal", [6, N_K * 512], BF16, isOutput=False)
    sc = nc.declare_dram_parameter("sc", [1, 11], F32, isOutput=False)
    aux = nc.declare_dram_parameter("aux", [128, N_WIN], F32, isOutput=False)
    offr = nc.declare_dram_parameter("offr", [128, FSH], F32, isOutput=False)
    srep = nc.declare_dram_parameter("srep", [128, tok], F32, isOutput=False)
    out = nc.declare_dram_parameter("o", [FSH, tok], F32, isOutput=True)

    DR = mybir.MatmulPerfMode.DoubleRow

    with tile.TileContext(nc) as tc:
        with (
            tc.tile_pool(name="const", bufs=1) as const,
            tc.tile_pool(name="wtpool", bufs=1) as wtpool,
            tc.tile_pool(name="xtp", bufs=1) as xtp,
            tc.tile_pool(name="ssb", bufs=1) as ssbp,
            tc.tile_pool(name="whp", bufs=4) as whp,
            tc.tile_pool(name="hfp", bufs=3) as hfp,
            tc.tile_pool(name="ob", bufs=3) as obp,
            tc.tile_pool(name="psw", bufs=4, space="PSUM") as psw,
            tc.tile_pool(name="psx", bufs=4, space="PSUM") as psx,
        ):
            # ---- small inputs (ACT hwdge ring, ahead of everything) -------
            sc_sb = const.tile([1, 11], F32)
            nc.scalar.dma_start(out=sc_sb[:], in_=sc[:])
            aux_sb = const.tile([128, N_WIN], F32)
            nc.scalar.dma_start(out=aux_sb[:], in_=aux[:])
            offr_sb = const.tile([128, FSH], F32)
            nc.scalar.dma_start(out=offr_sb[:], in_=offr[:])

            # ---- W^T slot chunk tiles, alternating between the sync and
            # ACT hwdge rings (each ring sustains only ~230GB/s for a single
            # transfer; two parallel streams keep the transform fed).  All 8
            # are queued up-front; with 4 bufs every pool gate is satisfied
            # long before the ring reaches the entry (no stalls). -----------
            wh_tiles = []
            for ci, (s0, ns) in enumerate(SLOT_CHUNKS):
                wch = whp.tile([128, ns, WTW], BF16, tag="wch", name=f"wch{ci}")
                wh_tiles.append((s0, ns, wch))
                eng = nc.sync if ci % 2 == 0 else nc.scalar
                eng.dma_start(out=wch[:], in_=wht[:, s0 * WTW:(s0 + ns) * WTW])

            # ---- halo-row chunk tiles (pre-packed [6, v, 512] on host) ----
            hal_tiles = []
            for ci in range(HAL_CHUNKS):
                hch = hfp.tile([6, 4, 512], BF16, tag="hal", name=f"hal{ci}")
                hal_tiles.append(hch)
                if ci < 2:
                    nc.scalar.dma_start(
                        out=hch[:],
                        in_=hal[:, 4 * ci * 512:4 * (ci + 1) * 512])

            # ---- bulk x^T: first half on the sync ring (behind the wht
            # chunks there); second half + S-replica issued after the
            # transform so they queue behind the ACT-ring transform stream --
            xt = xtp.tile([128, N_K, tok], F8)
            s_sb = ssbp.tile([128, tok], F32)
            for j in range(4):
                nc.sync.dma_start(
                    out=xt[:, 4 * j:4 * j + 4, :],
                    in_=xt_d[:, 4 * j * tok:(4 * j + 4) * tok])

            # ---- setup: broadcast scalars, band + halo matrices -----------
            ones_r = const.tile([1, 128], F32)
            nc.vector.memset(ones_r[:], 1.0)
            ps_b = psw.tile([128, 11], F32, tag="pw")
            nc.tensor.matmul(ps_b[:], ones_r[:], sc_sb[:], start=True, stop=True)
            scv = const.tile([128, 11], F32)
            nc.vector.tensor_copy(out=scv[:], in_=ps_b[:])

            # ctr = conv_w[h,1,1] + sigmoid(sk_wt[h])
            sig = const.tile([128, 1], F32)
            nc.scalar.activation(sig[:], scv[:, 10:11],
                                 mybir.ActivationFunctionType.Sigmoid)
            ctr = const.tile([128, 1], F32)
            nc.vector.tensor_tensor(out=ctr[:], in0=sig[:], in1=scv[:, 4:5],
                                    op=mybir.AluOpType.add)

            # band matrices B_dr[k, f] = cw[h, dr, k-f] (k-f = dc in {0,1,2});
            # the dr=1 matmul's center diagonal also carries the sigmoid
            # residual.  (Transposed space: contraction = shifted fin rows.)
            masks = []
            for d in range(3):
                m = const.tile([128, 128], F32, tag=f"mask{d}")
                nc.gpsimd.memset(m[:], 0.0)
                nc.gpsimd.affine_select(
                    out=m[:], in_=m[:],
                    compare_op=mybir.AluOpType.not_equal,
                    fill=1.0, base=-d, channel_multiplier=1,
                    pattern=[[-1, 128]],
                )
                masks.append(m)
            b_bf = []
            for dr in range(3):
                bf_ = const.tile([128, 128], F32, tag=f"bf_{dr}")
                nc.vector.tensor_scalar(bf_[:], masks[0][:],
                                        scv[:, 3 * dr:3 * dr + 1],
                                        None, mybir.AluOpType.mult)
                mid = ctr if dr == 1 else scv[:, 3 * dr + 1:3 * dr + 2]
                nc.vector.scalar_tensor_tensor(
                    out=bf_[:], in0=masks[1][:], scalar=mid, in1=bf_[:],
                    op0=mybir.AluOpType.mult, op1=mybir.AluOpType.add)
                nc.vector.scalar_tensor_tensor(
                    out=bf_[:], in0=masks[2][:],
                    scalar=scv[:, 3 * dr + 2:3 * dr + 3],
                    in1=bf_[:],
                    op0=mybir.AluOpType.mult, op1=mybir.AluOpType.add)
                bb = const.tile([128, 128], BF16, tag=f"bb_{dr}")
                nc.vector.tensor_copy(out=bb[:], in_=bf_[:])
                b_bf.append(bb)

            # halo matrix H6 [6, 128], j = row*3+dr over the 2 halo fin rows
            # x 3 fout shifts: col 127 takes center/dc=+1 taps from halo rows
            # 0/1, col 126 its dc=+1 tap from halo row 0.  Built as outer
            # products (engine APs cannot start at a nonzero partition).
            onehot = const.tile([1, 128], F32)
            nc.vector.memset(onehot[:], 0.0)
            nc.vector.memset(onehot[:, 127:128], 1.0)
            onehot6 = const.tile([1, 128], F32)
            nc.vector.memset(onehot6[:], 0.0)
            nc.vector.memset(onehot6[:, 126:127], 1.0)
            sig0 = const.tile([1, 1], F32)
            nc.scalar.activation(sig0[:], sc_sb[:, 10:11],
                                 mybir.ActivationFunctionType.Sigmoid)
            ctr0 = const.tile([1, 1], F32)
            nc.vector.tensor_tensor(out=ctr0[:], in0=sig0[:], in1=sc_sb[:, 4:5],
                                    op=mybir.AluOpType.add)
            # v1 (col 127): row0 -> cw[dr,1] (ctr at dr=1), row1 -> cw[dr,2]
            v1 = const.tile([1, 6], F32)
            nc.vector.tensor_copy(out=v1[:, 0:1], in_=sc_sb[:, 1:2])
            nc.vector.tensor_copy(out=v1[:, 1:2], in_=ctr0[:])
            nc.vector.tensor_copy(out=v1[:, 2:3], in_=sc_sb[:, 7:8])
            nc.vector.tensor_copy(out=v1[:, 3:4], in_=sc_sb[:, 2:3])
            nc.vector.tensor_copy(out=v1[:, 4:5], in_=sc_sb[:, 5:6])
            nc.vector.tensor_copy(out=v1[:, 5:6], in_=sc_sb[:, 8:9])
            # v2 (col 126): row0 -> cw[dr,2]
            v2 = const.tile([1, 6], F32)
            nc.vector.memset(v2[:], 0.0)
            nc.vector.tensor_copy(out=v2[:, 0:1], in_=sc_sb[:, 2:3])
            nc.vector.tensor_copy(out=v2[:, 1:2], in_=sc_sb[:, 5:6])
            nc.vector.tensor_copy(out=v2[:, 2:3], in_=sc_sb[:, 8:9])
            ph = psw.tile([6, 128], F32, tag="pw")
            nc.tensor.matmul(ph[:], v1[:], onehot[:], start=True, stop=False)
            nc.tensor.matmul(ph[:], v2[:], onehot6[:], start=False, stop=True)
            h6 = const.tile([6, 128], BF16)
            nc.vector.tensor_copy(out=h6[:], in_=ph[:])

            wt = wtpool.tile([128, N_K, FSH], F8)      # W_i^T, fin-major, fp8

            # ---- phase T: transposed weight transform ---------------------
            for ci, (s0, ns, wch) in enumerate(wh_tiles):
                for vi in range(ns):
                    v = s0 + vi
                    if v % 4 == 2 and 2 <= v // 4 + 2 < HAL_CHUNKS:
                        c2 = v // 4 + 2
                        nc.scalar.dma_start(
                            out=hal_tiles[c2][:],
                            in_=hal[:, 4 * c2 * 512:4 * (c2 + 1) * 512])
                    pw = psw.tile([128, 512], F32, tag="pw")
                    for dr in range(3):
                        nc.tensor.matmul(
                            pw[:], b_bf[dr][:], wch[:, vi, dr:dr + 512],
                            start=(dr == 0), stop=False)
                    nc.tensor.matmul(pw[:], h6[:],
                                     hal_tiles[v // 4][:, v % 4, :],
                                     start=False, stop=True)
                    # PSUM -> fp8 W_i^T slot with (conv_b - mu)[o] column add
                    nc.vector.tensor_tensor(
                        out=wt[:, v, :], in0=pw[:], in1=offr_sb[:],
                        op=mybir.AluOpType.add)

            # second x^T half + S-replica on the ACT ring, after the
            # transform stream
            for j in (4, 5):
                nc.scalar.dma_start(
                    out=xt[:, 4 * j:4 * j + 4, :],
                    in_=xt_d[:, 4 * j * tok:(4 * j + 4) * tok])
            nc.scalar.dma_start(out=s_sb[:], in_=srep[:])
            for j in (6, 7):
                nc.scalar.dma_start(
                    out=xt[:, 4 * j:4 * j + 4, :],
                    in_=xt_d[:, 4 * j * tok:(4 * j + 4) * tok])

            # ---- phase M: fp8 DoubleRow matmul, W_i^T chunks stationary ---
            # out^T[oc*128 + p, t]; k-inner per PSUM bank (no bank switch
            # between accumulating matmuls).
            for oc in range(N_WIN):
                pos = []
                for b in range(n_tb):
                    pool, tg = (psw, "pw") if b % 2 == 0 else (psx, "px")
                    pos.append(pool.tile([128, 512], F32, tag=tg,
                                         name=f"po{b}"))
                for b in range(n_tb):
                    for k in range(n_kp):
                        if "no_mm" in probe:
                            continue
                        nc.tensor.matmul(
                            pos[b][:],
                            wt[:, 2 * k:2 * k + 2, 128 * oc:128 * oc + 128],
                            xt[:, 2 * k:2 * k + 2, 512 * b:512 * b + 512],
                            start=(k == 0), stop=(k == n_kp - 1),
                            perf_mode=DR)
                    if "no_mm" in probe:
                        nc.vector.memset(pos[b][:], 0.0)
                    ob = obp.tile([128, 512], F32, tag="ob")
                    # ob = psum + mu[o] * S[t]  (rank-1 mean compensation)
                    nc.vector.scalar_tensor_tensor(
                        out=ob[:], in0=s_sb[:, 512 * b:512 * b + 512],
                        scalar=aux_sb[:, oc:oc + 1],
                        in1=pos[b][:],
                        op0=mybir.AluOpType.mult, op1=mybir.AluOpType.add)
                    nc.sync.dma_start(
                        out=out[128 * oc:128 * oc + 128,
                                512 * b:512 * b + 512],
                        in_=ob[:])

    nc.compile()
    return nc


def _host_transform_f32(W, conv_w, conv_b, sk_wt):
    """f32 numpy replica of the reference weight transform (used only to
    derive the per-row means mu for fp8 mean-compensation)."""
    hsz = FOUT // NUM_HEADS
    mat = W.reshape(NUM_HEADS, hsz, FIN)
    sig = (1.0 / (1.0 + np.exp(-sk_wt.astype(np.float64)))).astype(np.float32)
    sig = sig.reshape(NUM_HEADS)
    out = np.empty_like(mat)
    for h in range(NUM_HEADS):
        p = np.pad(mat[h], ((1, 1), (1, 1)))
        acc = np.zeros((hsz, FIN), dtype=np.float32)
        for dr in range(3):
            for dc in range(3):
                acc += conv_w[h, 0, dr, dc] * p[dr:dr + hsz, dc:dc + FIN]
        out[h] = acc + conv_b[h] + sig[h] * mat[h]
    return out.reshape(FOUT, FIN)


def shard_inputs(inp, W, conv_w, conv_b, sk_wt):
    """Build the 8 per-core input maps."""
    x = np.ascontiguousarray(np.asarray(inp, dtype=np.float32).reshape(TOK, FIN))
    W = np.asarray(W, dtype=np.float32)
    conv_w = np.asarray(conv_w, dtype=np.float32)
    conv_b = np.asarray(conv_b, dtype=np.float32).reshape(NUM_HEADS)
    sk_wt = np.asarray(sk_wt, dtype=np.float32)

    # fp8 x^T in k-block layout: xt[p, k, t] = fp8(x[t, 128k + p])
    x8T = np.ascontiguousarray(x.astype(NP_F8).T)          # [fin, tok]
    xt_host = np.ascontiguousarray(
        x8T.reshape(N_K, 128, TOK).transpose(1, 0, 2)).reshape(128, N_K * TOK)

    # exact f32 row sums of x, replicated across partitions
    srow = x.astype(np.float64).sum(axis=1).astype(np.float32)
    srep = np.ascontiguousarray(
        np.broadcast_to(srow[None, :], (128, TOK)))

    # per-row means of W_i (host f32 replica; quantization statistic only)
    W_i = _host_transform_f32(W, conv_w, conv_b, sk_wt)
    mu = W_i.astype(np.float64).mean(axis=1).astype(np.float32)  # [FOUT]

    hsz = FOUT // NUM_HEADS
    in_maps = []
    for c in range(NCORES):
        gr0 = c * FSH
        h = (gr0 // hsz) % NUM_HEADS
        # W^T halo slab: rows = fout gr0-1 .. gr0+512 (zero-padded at the
        # head boundary), cols = all fin.  -> [514, FIN] bf16
        slab = np.zeros((FSH + 2, FIN), dtype=NP_BF16)
        lo = max(gr0 - 1, h * hsz)
        hi = min(gr0 + FSH + 1, (h + 1) * hsz)
        slab[lo - (gr0 - 1):hi - (gr0 - 1), :] = W[lo:hi, :].astype(NP_BF16)
        slabT = np.ascontiguousarray(slab.T)               # [FIN, 514]
        # slots: wht[p, v, :] = W^T row (128v + p - 1), zero at fin edges
        rows = np.zeros((128 * N_SLOT, WTW), dtype=NP_BF16)
        for v in range(N_SLOT):
            r0 = 128 * v - 1
            a = max(r0, 0)
            bnd = min(r0 + 128, FIN)
            if a < bnd:
                rows[128 * v + (a - r0):128 * v + (bnd - r0), :] = slabT[a:bnd]
        whT_host = np.ascontiguousarray(
            rows.reshape(N_SLOT, 128, WTW).transpose(1, 0, 2)
        ).reshape(128, N_SLOT * WTW)
        # halo rows: hal[r*3+dr, v, t] = W^T row (128v + 127 + r), col dr+t
        slabT_pad = np.vstack([slabT, np.zeros((1, WTW), dtype=NP_BF16)])
        hal6 = np.zeros((6, N_K, 512), dtype=NP_BF16)
        ridx = 128 * np.arange(N_K) + 127
        for r in range(2):
            for dr in range(3):
                hal6[r * 3 + dr] = slabT_pad[ridx + r, dr:dr + 512]
        hal_host = np.ascontiguousarray(hal6).reshape(6, N_K * 512)

        scal = np.zeros((1, 11), dtype=np.float32)
        scal[0, :9] = conv_w[h].reshape(9)
        scal[0, 9] = conv_b[h]
        scal[0, 10] = np.float32(sk_wt[h].reshape(()))
        muc = mu[gr0:gr0 + FSH]
        auxm = np.ascontiguousarray(muc.reshape(N_WIN, 128).T)   # [128, 4]
        offr = np.ascontiguousarray(np.broadcast_to(
            (conv_b[h] - muc)[None, :], (128, FSH))).astype(np.float32)
        in_maps.append({"xt": xt_host, "wht": whT_host, "hal": hal_host,
                        "sc": scal, "aux": auxm, "offr": offr, "srep": srep})
    return in_maps


_PROGRAM_CACHE = {}


def _get_program(tok=TOK, probe=()):
    key = (tok, tuple(probe))
    if key not in _PROGRAM_CACHE:
        _PROGRAM_CACHE[key] = build_program(tok, probe)
    return _PROGRAM_CACHE[key]


def kernel(inp, W, conv_w, conv_b, sk_wt):
    nc = _get_program(TOK)
    in_maps = shard_inputs(inp, W, conv_w, conv_b, sk_wt)
    res = run_bass_kernel_spmd(nc, in_maps, list(range(NCORES)))
    outT = np.concatenate([res.results[c]["o"] for c in range(NCORES)],
                          axis=0)                      # [FOUT, TOK]
    return np.ascontiguousarray(outT.T.astype(np.float32)
                                ).reshape(2, TOK // 2, FOUT)
